# revision 6
# baseline (speedup 1.0000x reference)
"""Self-contained Trainium2 (8-core) kernel for the 3-layer AdjGCN.

Math (matches reference):
  A = D^-1/2 (Adj + I) D^-1/2  built from edge_index with self-loops
  h1 = relu(A @ (x @ W1) + b1)
  h2 = relu((A @ h1) @ W2 + b2)        == relu(A @ (h1 @ W2') ... NO: uses assoc:
       (A @ h1) @ W2 == A @ (h1 @ W2)  (associativity)
  out = A @ (h2 @ W3) + b3

Distribution: destination nodes sharded across 8 cores. Per core, nodes are
packed into tiles of 128 (partition dim) balancing per-subtable edge counts.
Edges are grouped by (dest-tile-pair, source-subtable) for int16 dma_gather
from the AllGather'd activation table (4 subtables of 2*NPAD < 32768 rows).
Aggregation per 128-edge chunk is a TensorE matmul with an on-device-built
selection matrix S[e, dest_local] = wn[e]. Self-loops are applied as one
extra diagonal chunk reading the core's own (pre-AllGather) activations.
"""

import numpy as np
import ml_dtypes

CORES = 8
P = 128

BF16 = ml_dtypes.bfloat16


# ---------------------------------------------------------------- host prep

def _pack_nodes(d_vs_core, tpc, cap_iter=4000):
    """Pack npc nodes (rows of d_vs_core [npc, 4] subtable in-degrees) into
    tpc tiles of <=128 slots, minimizing max per-(tile, subtable) edge count.
    Returns (tiles: list of node-index arrays (core-local), e_ts [tpc, 4])."""
    npc = d_vs_core.shape[0]
    tot = d_vs_core.sum(1)
    order = np.argsort(-tot, kind="stable")
    # serpentine deal
    assign = np.empty(npc, np.int32)
    nrounds = (npc + tpc - 1) // tpc
    pos = 0
    for r in range(nrounds):
        k = min(tpc, npc - pos)
        idx = order[pos:pos + k]
        bins = np.arange(k) if r % 2 == 0 else (tpc - 1 - np.arange(k))
        assign[idx] = bins
        pos += k
    counts = np.bincount(assign, minlength=tpc)
    assert counts.max() <= P
    e_ts = np.zeros((tpc, 4), np.int64)
    np.add.at(e_ts, assign, d_vs_core)

    # repair: push the max (t,s) down by moving/swapping its heaviest nodes
    mean = e_ts.mean()
    target = int(np.ceil((mean + 3.0 * max(e_ts.std(), 1.0)) / P) * P)
    for _ in range(cap_iter):
        t, s = np.unravel_index(np.argmax(e_ts), e_ts.shape)
        if e_ts[t, s] <= target:
            break
        members = np.where(assign == t)[0]
        v = members[np.argmax(d_vs_core[members, s])]
        t2 = int(np.argmin(e_ts[:, s]))
        if t2 == t:
            break
        if counts[t2] < P:
            assign[v] = t2
            counts[t] -= 1
            counts[t2] += 1
            e_ts[t] -= d_vs_core[v]
            e_ts[t2] += d_vs_core[v]
        else:
            m2 = np.where(assign == t2)[0]
            u = m2[np.argmin(d_vs_core[m2, s])]
            if d_vs_core[u, s] >= d_vs_core[v, s]:
                break
            assign[v] = t2
            assign[u] = t
            e_ts[t] += d_vs_core[u] - d_vs_core[v]
            e_ts[t2] += d_vs_core[v] - d_vs_core[u]
    tiles = [np.where(assign == t)[0] for t in range(tpc)]
    return tiles, e_ts, assign


def prep(x, edge_index, W1, b1, W2, b2, W3, b3):
    """Host preprocessing: sharding, packing, edge slot assignment.
    Returns (meta dict, per-core input maps list)."""
    N = x.shape[0]
    assert N % CORES == 0
    npc = N // CORES
    tpc = -(-npc // P)
    if tpc % 2:
        tpc += 1
    npad = tpc * P
    ng = tpc // 2
    subrows = 2 * npad
    assert subrows <= 32768

    row = np.asarray(edge_index[0], np.int64)
    col = np.asarray(edge_index[1], np.int64)
    E = row.shape[0]

    deg = np.bincount(row, minlength=N).astype(np.float64) + 1.0
    dinv = (deg ** -0.5).astype(np.float32)
    wn_e = dinv[row] * dinv[col]
    wn_self = (dinv * dinv).astype(np.float32)

    core_of = (row // npc).astype(np.int32)
    s_of = (col // (2 * npc)).astype(np.int32)

    # per-node subtable in-degree (random edges only)
    d_vs = np.zeros(N * 4, np.int64)
    np.add.at(d_vs, row * 4 + s_of, 1)
    d_vs = d_vs.reshape(N, 4)

    # pack per core
    perm = np.full((CORES, npad), -1, np.int64)      # slot -> global node
    pos_of = np.full(N, -1, np.int32)                # node -> slot in its core
    e_ts_all = np.zeros((CORES, tpc, 4), np.int64)
    for c in range(CORES):
        dc = d_vs[c * npc:(c + 1) * npc]
        tiles, e_ts, _ = _pack_nodes(dc, tpc)
        e_ts_all[c] = e_ts
        for t, members in enumerate(tiles):
            g = members + c * npc
            perm[c, t * P: t * P + len(g)] = g
            pos_of[g] = t * P + np.arange(len(g), dtype=np.int32)

    cps = int(-(-e_ts_all.max() // P))
    ch = 4 * cps + 1           # chunks per tile (self chunk first)
    ni = 2 * cps * P           # gather indices per (pair, subtable) instr
    ni16 = ni // 16

    F_h = W1.shape[1]
    F_o = W3.shape[1]

    # edge destination slot info
    e_pos = pos_of[row]                  # slot of dest within core [0, npad)
    e_tile = e_pos // P
    e_dl = (e_pos % P).astype(np.int32)  # dest local partition
    # source table local index within subtable
    c_src = (col // npc).astype(np.int64)
    e_loc16 = ((c_src % 2) * npad + pos_of[col]).astype(np.int64)

    in_maps = []
    for c in range(CORES):
        m = core_of == c
        r_t = e_tile[m]
        r_s = s_of[m]
        r_dl = e_dl[m]
        r_loc = e_loc16[m]
        r_wn = wn_e[m]
        # rank within (tile, subtable)
        key = r_t * 4 + r_s
        order = np.argsort(key, kind="stable")
        key_s = key[order]
        starts = np.searchsorted(key_s, np.arange(tpc * 4))
        rank = np.arange(len(key_s)) - starts[key_s]
        kslots = cps * P
        assert rank.max() < kslots, (rank.max(), kslots)
        # slot within its (t,s) block
        t_o, s_o_, dl_o, loc_o, wn_o = (
            r_t[order], r_s[order], r_dl[order], r_loc[order], r_wn[order])

        # dl / wn arrays [128, tpc*ch]
        dl_arr = np.zeros((P, tpc * ch), np.float32)
        wn_arr = np.zeros((P, tpc * ch), np.float32)
        # self chunk k=0 per tile: dl = iota, wn = wn_self (valid slots only)
        iota = np.arange(P, dtype=np.float32)
        for t in range(tpc):
            colk = t * ch
            dl_arr[:, colk] = iota
            slots = perm[c, t * P:(t + 1) * P]
            valid = slots >= 0
            wn_arr[valid, colk] = wn_self[slots[valid]]
        # gather chunks: chunk index k = 1 + s*cps + (rank // 128)
        kk = 1 + s_o_ * cps + rank // P
        ee = rank % P
        cols_ = t_o * ch + kk
        dl_arr[ee, cols_] = dl_o.astype(np.float32)
        wn_arr[ee, cols_] = wn_o

        # idx16 [128, ng*4*ni16]
        ids_all = np.zeros((ng * 4, ni), np.int64)  # instr, i
        gi = (t_o // 2) * 4 + s_o_                   # instr id
        ci = (t_o % 2) * cps + rank // P             # chunk-in-instr
        ii = ci * P + ee
        ids_all[gi, ii] = loc_o
        wrapped = ids_all.reshape(ng * 4, ni16, 16).transpose(0, 2, 1)  # [inst, 16, ni16]
        idx16 = np.tile(wrapped, (1, 8, 1)).transpose(1, 0, 2).reshape(P, ng * 4 * ni16)
        idx16 = np.ascontiguousarray(idx16.astype(np.int16))

        # x_t [F_IN, npad]
        xs = np.zeros((npad, x.shape[1]), np.float32)
        valid = perm[c] >= 0
        xs[valid] = np.asarray(x)[perm[c][valid]]
        x_t = np.ascontiguousarray(xs.T.astype(BF16))

        in_maps.append({
            "x_t": x_t,
            "idx16": idx16,
            "dl": np.ascontiguousarray(dl_arr.astype(BF16)),
            "wn": np.ascontiguousarray(wn_arr.astype(BF16)),
            "w1": np.ascontiguousarray(
                np.asarray(W1, np.float32).reshape(-1, P, F_h).transpose(1, 0, 2)
                .reshape(P, -1).astype(BF16)),
            "w2": np.ascontiguousarray(
                np.asarray(W2, np.float32).reshape(-1, P, F_h).transpose(1, 0, 2)
                .reshape(P, -1).astype(BF16)),
            "w3": np.ascontiguousarray(
                np.asarray(W3, np.float32).reshape(-1, P, F_o).transpose(1, 0, 2)
                .reshape(P, -1).astype(BF16)),
            "bias1": np.ascontiguousarray(np.tile(np.asarray(b1, np.float32), (P, 1))),
            "bias2": np.ascontiguousarray(np.tile(np.asarray(b2, np.float32), (P, 1))),
            "bias3": np.ascontiguousarray(np.tile(np.asarray(b3, np.float32), (P, 1))),
            "iota": np.ascontiguousarray(
                np.tile(np.arange(P, dtype=np.float32), (P, 1)).astype(BF16)),
        })

    meta = dict(N=N, npc=npc, tpc=tpc, npad=npad, ng=ng, cps=cps, ch=ch,
                ni=ni, ni16=ni16, F_in=x.shape[1], F_h=F_h, F_o=F_o,
                perm=perm)
    return meta, in_maps


# ---------------------------------------------------------------- device

def build(meta, bench_k=0, use_collectives=True):
    """bench_k=0: normal. bench_k>=1: wrap the whole pipeline in For_i(bench_k)
    for loop-slope timing (requires use_collectives=False; AllGathers replaced
    by a small dependency-preserving copy)."""
    import concourse.bass as bass
    import concourse.tile as tile
    from concourse import bacc, mybir
    from concourse.masks import make_identity

    tpc, ng, cps, ch = meta["tpc"], meta["ng"], meta["cps"], meta["ch"]
    ni, ni16 = meta["ni"], meta["ni16"]
    npad = meta["npad"]
    F_in, F_h, F_o = meta["F_in"], meta["F_h"], meta["F_o"]
    subrows = 2 * npad
    dt = mybir.dt
    op = mybir.AluOpType

    nc = bacc.Bacc("TRN2", target_bir_lowering=False, debug=False,
                   num_devices=CORES, num_swdge_queues=4)
    x_t = nc.dram_tensor("x_t", [F_in, npad], dt.bfloat16, kind="ExternalInput")
    idx16 = nc.dram_tensor("idx16", [P, ng * 4 * ni16], dt.int16, kind="ExternalInput")
    dl = nc.dram_tensor("dl", [P, tpc * ch], dt.bfloat16, kind="ExternalInput")
    wn = nc.dram_tensor("wn", [P, tpc * ch], dt.bfloat16, kind="ExternalInput")
    w1 = nc.dram_tensor("w1", [P, (F_in // P) * F_h], dt.bfloat16, kind="ExternalInput")
    w2 = nc.dram_tensor("w2", [P, (F_h // P) * F_h], dt.bfloat16, kind="ExternalInput")
    w3 = nc.dram_tensor("w3", [P, (F_h // P) * F_o], dt.bfloat16, kind="ExternalInput")
    bias1 = nc.dram_tensor("bias1", [P, F_h], dt.float32, kind="ExternalInput")
    bias2 = nc.dram_tensor("bias2", [P, F_h], dt.float32, kind="ExternalInput")
    bias3 = nc.dram_tensor("bias3", [P, F_o], dt.float32, kind="ExternalInput")
    iota_in = nc.dram_tensor("iota", [P, P], dt.bfloat16, kind="ExternalInput")
    out = nc.dram_tensor("out", [npad, F_o], dt.float32, kind="ExternalOutput")

    bounce0 = nc.dram_tensor("bounce0", [npad, F_h], dt.bfloat16)
    bounce1 = nc.dram_tensor("bounce1", [npad, F_h], dt.bfloat16)
    bounce2 = nc.dram_tensor("bounce2", [npad, F_o], dt.bfloat16)
    T0 = nc.dram_tensor("T0", [CORES * npad, F_h], dt.bfloat16, addr_space="Shared")
    T1 = nc.dram_tensor("T1", [CORES * npad, F_h], dt.bfloat16, addr_space="Shared")
    T2 = nc.dram_tensor("T2", [CORES * npad, F_o], dt.bfloat16, addr_space="Shared")
    rg = [list(range(CORES))]

    with tile.TileContext(nc) as tc:
        import contextlib
        with contextlib.ExitStack() as ctx:
            cpool = ctx.enter_context(tc.tile_pool(name="consts", bufs=1))
            gpool = ctx.enter_context(tc.tile_pool(name="feat", bufs=2))
            spool = ctx.enter_context(tc.tile_pool(name="sel", bufs=2))
            ppool = ctx.enter_context(tc.tile_pool(name="psum", bufs=2, space="PSUM"))
            hpool = ctx.enter_context(tc.tile_pool(name="work", bufs=3))

            # ---- resident constants
            idx_sb = cpool.tile([P, ng * 4 * ni16], dt.int16)
            nc.sync.dma_start(out=idx_sb[:], in_=idx16[:])
            dl_sb = cpool.tile([P, tpc * ch], dt.bfloat16)
            nc.sync.dma_start(out=dl_sb[:], in_=dl[:])
            wn_sb = cpool.tile([P, tpc * ch], dt.bfloat16)
            nc.sync.dma_start(out=wn_sb[:], in_=wn[:])
            iota_sb = cpool.tile([P, P], dt.bfloat16)
            nc.sync.dma_start(out=iota_sb[:], in_=iota_in[:])
            w1_sb = cpool.tile([P, (F_in // P) * F_h], dt.bfloat16)
            nc.sync.dma_start(out=w1_sb[:], in_=w1[:])
            w2_sb = cpool.tile([P, (F_h // P) * F_h], dt.bfloat16)
            nc.sync.dma_start(out=w2_sb[:], in_=w2[:])
            w3_sb = cpool.tile([P, (F_h // P) * F_o], dt.bfloat16)
            nc.sync.dma_start(out=w3_sb[:], in_=w3[:])
            b1_sb = cpool.tile([P, F_h], dt.float32)
            nc.sync.dma_start(out=b1_sb[:], in_=bias1[:])
            b2_sb = cpool.tile([P, F_h], dt.float32)
            nc.sync.dma_start(out=b2_sb[:], in_=bias2[:])
            b3_sb = cpool.tile([P, F_o], dt.float32)
            nc.sync.dma_start(out=b3_sb[:], in_=bias3[:])
            ident_sb = cpool.tile([P, P], dt.bfloat16)
            make_identity(nc, ident_sb[:])

            def allgather(bounce, T):
                if use_collectives:
                    nc.gpsimd.collective_compute(
                        "AllGather", op.bypass, ins=[bounce[:]], outs=[T[:]],
                        replica_groups=rg)
                else:
                    nc.sync.dma_start(out=T[0:P, :], in_=bounce[0:P, :])

            # ---- phase 0: bounce0 = x @ W1 (per tile)
            kin = F_in // P

            def phase0():
                for t in range(tpc):
                    ps = ppool.tile([P, F_h], dt.float32, tag="psA")
                    for i in range(kin):
                        xt = hpool.tile([P, P], dt.bfloat16, tag="xt")
                        nc.sync.dma_start(
                            out=xt[:], in_=x_t[i * P:(i + 1) * P, t * P:(t + 1) * P])
                        nc.tensor.matmul(ps[:], lhsT=xt[:],
                                         rhs=w1_sb[:, i * F_h:(i + 1) * F_h],
                                         start=(i == 0), stop=(i == kin - 1))
                    h0 = hpool.tile([P, F_h], dt.bfloat16, tag="hcast")
                    nc.vector.tensor_copy(h0[:], ps[:])
                    nc.sync.dma_start(out=bounce0[t * P:(t + 1) * P, :], in_=h0[:])

            # ---- spmm phases
            def spmm(T, F, bprev, bias_sb, relu, dense, bout, Fn):
                for g in range(ng):
                    feat = gpool.tile([P, 4 * 2 * cps * F], dt.bfloat16, tag="feat")
                    fv = feat[:].rearrange("p (s c f) -> p s c f", s=4, f=F)
                    for s in range(4):
                        nc.gpsimd.dma_gather(
                            out_ap=fv[:, s, :, :],
                            in_ap=T[s * subrows:(s + 1) * subrows, :],
                            idxs_ap=idx_sb[:, (g * 4 + s) * ni16:(g * 4 + s + 1) * ni16],
                            num_idxs=ni, num_idxs_reg=ni, elem_size=F,
                            single_packet=False, queue_num=(g * 4 + s) % 4)
                    for dt_ in range(2):
                        t = 2 * g + dt_
                        sf = hpool.tile([P, F], dt.bfloat16, tag="selffeat")
                        nc.sync.dma_start(out=sf[:], in_=bprev[t * P:(t + 1) * P, :])
                        S = spool.tile([P, ch * P], dt.bfloat16, tag="S")
                        sv = S[:].rearrange("p (k j) -> p k j", j=P)
                        dlb = dl_sb[:, t * ch:(t + 1) * ch].unsqueeze(2).to_broadcast([P, ch, P])
                        wnb = wn_sb[:, t * ch:(t + 1) * ch].unsqueeze(2).to_broadcast([P, ch, P])
                        iob = iota_sb[:].unsqueeze(1).to_broadcast([P, ch, P])
                        nc.vector.tensor_tensor(out=sv[:], in0=dlb, in1=iob, op=op.is_equal)
                        nc.vector.tensor_tensor(out=sv[:], in0=sv[:], in1=wnb, op=op.mult)
                        ps = ppool.tile([P, F], dt.float32, tag="psA")
                        nc.tensor.matmul(ps[:], lhsT=S[:, 0:P], rhs=sf[:],
                                         start=True, stop=False)
                        for s in range(4):
                            for cci in range(cps):
                                k = 1 + s * cps + cci
                                ci = dt_ * cps + cci
                                nc.tensor.matmul(
                                    ps[:], lhsT=S[:, k * P:(k + 1) * P],
                                    rhs=fv[:, s, ci, :],
                                    start=False, stop=(k == ch - 1))
                        if dense is None:
                            o = hpool.tile([P, F_o], dt.float32, tag="outt")
                            nc.vector.tensor_tensor(out=o[:], in0=ps[:], in1=bias_sb[:], op=op.add)
                            nc.sync.dma_start(out=out[t * P:(t + 1) * P, :], in_=o[:])
                        else:
                            hb = hpool.tile([P, F], dt.bfloat16, tag="hb")
                            nc.vector.tensor_tensor(out=hb[:], in0=ps[:], in1=bias_sb[:], op=op.add)
                            hr = hpool.tile([P, F], dt.bfloat16, tag="hr")
                            nc.vector.tensor_scalar(out=hr[:], in0=hb[:], scalar1=0.0,
                                                    scalar2=None, op0=op.max)
                            wsb = dense
                            psd = ppool.tile([P, Fn], dt.float32, tag="psD")
                            for i in range(F // P):
                                tp = ppool.tile([P, P], dt.bfloat16, tag="psT")
                                nc.tensor.transpose(tp[:], hr[:, i * P:(i + 1) * P], ident_sb[:])
                                hT = hpool.tile([P, P], dt.bfloat16, tag="hT")
                                nc.scalar.copy(out=hT[:], in_=tp[:])
                                nc.tensor.matmul(psd[:], lhsT=hT[:],
                                                 rhs=wsb[:, i * Fn:(i + 1) * Fn],
                                                 start=(i == 0), stop=(i == F // P - 1))
                            ho = hpool.tile([P, Fn], dt.bfloat16, tag="hcast")
                            nc.vector.tensor_copy(ho[:], psd[:])
                            nc.sync.dma_start(out=bout[t * P:(t + 1) * P, :], in_=ho[:])

            def pipeline():
                phase0()
                allgather(bounce0, T0)
                spmm(T0, F_h, bounce0, b1_sb, True, w2_sb, bounce1, F_h)
                allgather(bounce1, T1)
                spmm(T1, F_h, bounce1, b2_sb, True, w3_sb, bounce2, F_o)
                allgather(bounce2, T2)
                spmm(T2, F_o, bounce2, b3_sb, False, None, None, None)

            if bench_k >= 1:
                assert not use_collectives
                with tc.For_i(0, bench_k, 1):
                    pipeline()
            else:
                pipeline()

    nc.compile()
    return nc


def kernel(x, edge_index, W1, b1, W2, b2, W3, b3):
    from concourse.bass_utils import run_bass_kernel_spmd

    meta, in_maps = prep(x, edge_index, W1, b1, W2, b2, W3, b3)
    nc = build(meta)
    res = run_bass_kernel_spmd(nc, in_maps, list(range(CORES)), trace=False)
    N, F_o = meta["N"], meta["F_o"]
    perm = meta["perm"]
    outp = np.empty((N, F_o), np.float32)
    for c in range(CORES):
        shard = res.results[c]["out"]
        valid = perm[c] >= 0
        outp[perm[c][valid]] = shard[valid]
    return outp
